# revision 27
# baseline (speedup 1.0000x reference)
"""GAT (4-layer, 8-head) Trainium2 kernel, 8-core SPMD.

Strategy (per sharding hint): nodes partitioned into 8 contiguous shards
(4096 nodes/core); each core owns the edges whose dst falls in its shard
(segment softmax + scatter stay local). Per-layer, each core computes the
dense ft = h @ W for its shard; the shard's packed rows [ft | el | er] are
then AllGathered so every core can fetch remote src rows with dma_gather.
The AllGather is split into P=2 concurrent halves (measured ~3x faster than
one big collective); edges are grouped per (dst-block, src-half) so each
half gathers from its own table. Edge aggregation (segment softmax +
weighted scatter-add) runs as PE matmuls against host-precomputed one-hot
dst matrices, with exp() folded into the moving operand. The dst-side er
is fetched from the LOCAL pre-AllGather table, so those gathers overlap
the collectives. Weights are replicated (and cast to bf16 on host).

Softmax: the reference subtracts a per-segment max; alpha is invariant to
constant shifts and the logit range here is [-2, 7], so we exp() directly
and normalize by the segment sum at the node side.
"""

import functools

import numpy as np

import concourse.bacc as bacc
import concourse.bass as bass
import concourse.mybir as mybir
import concourse.tile as tile
from concourse.bass_utils import run_bass_kernel_spmd

# ---- problem constants (hardcoded per contract) ----
N, E, G = 32768, 262144, 64
NCORES = 8
SH = N // NCORES          # 4096 nodes per core
NB = SH // 128            # 32 dst blocks per core
NCH = NB                  # node chunks per core (same 128-partition blocks)
F0, F = 128, 512
H, D = 8, 64
H3 = 6
NEG_SLOPE = 0.2
EPS = 1e-30

P = 1                     # AllGather split factor (1: fewer, bigger collectives won on HW)
SHP = SH // P             # shard rows per group
NCHP = NCH // P           # dense chunks per group
NT = N // P               # rows per gathered table

f32 = mybir.dt.float32
f32r = mybir.dt.float32r
bf16 = mybir.dt.bfloat16
fp8 = mybir.dt.float8e3
i16 = mybir.dt.int16

# dtype of the gathered ft tables / one-hot S / edge moving operand
FT_DT = bf16
DENSE_BF = False          # bf16 dense weights/activations
PK_FP8 = True             # fp8(e3m4) ft payload in the gathered tables
FT_SCALE = 2.0            # prescale ft before fp8 cast (dodges subnormal flush;
                          # e3m4 max normal ~15.5, |ft|max ~5.5 -> x2 is safe)
AG_LADDER = False         # recursive-doubling AllGather (3x 2-rank rounds)

TRACE = False
TRACE_KW = {}
LAST = {}

AF = mybir.ActivationFunctionType
ALU = mybir.AluOpType
AX = mybir.AxisListType


def _wrap_idx(v):
    """int16 gather-index layout: element i at [i%16, i//16], replicated to
    128 partitions."""
    L = len(v)
    w = np.zeros((16, L // 16), np.int16)
    w[np.arange(L) % 16, np.arange(L) // 16] = v.astype(np.int16)
    return np.tile(w, (8, 1))


def _bf(x):
    import ml_dtypes
    return np.asarray(x, np.float32).astype(ml_dtypes.bfloat16)


def preprocess(inputs):
    src = np.asarray(inputs["src"]).astype(np.int64)
    dst = np.asarray(inputs["dst"]).astype(np.int64)
    graph_id = np.asarray(inputs["graph_id"]).astype(np.int64)
    feat = np.asarray(inputs["feat"], dtype=np.float32)

    # pass 1: per-core, per-block, per-src-group edge lists + global chunk max
    per_core = []
    cnt_max = np.zeros(P, np.int64)
    for c in range(NCORES):
        m = (dst >= c * SH) & (dst < (c + 1) * SH)
        es, ed = src[m], dst[m]
        o = np.argsort(ed, kind="stable")
        es, ed = es[o], ed[o]
        dl = ed - c * SH
        blk = dl >> 7
        grp = (es % SH) // SHP
        blocks = []
        for b in range(NB):
            mb = blk == b
            eb, db = es[mb], dl[mb]
            gb = grp[mb]
            parts = []
            for p in range(P):
                mp = gb == p
                parts.append((eb[mp], db[mp]))
                cnt_max[p] = max(cnt_max[p], mp.sum())
            blocks.append(parts)
        per_core.append(blocks)
    KBS = tuple(int(np.ceil(c / 128)) for c in cnt_max)
    EBS = [k * 128 for k in KBS]
    KBT = sum(KBS)
    EBT = KBT * 128
    OFF = np.concatenate([[0], np.cumsum(KBS)]).astype(int)

    import ml_dtypes
    ft_np = (np.float32 if FT_DT == f32 else ml_dtypes.bfloat16)

    # shared weight-derived arrays
    def Amat(al):  # [1,H,D] -> [H*D, H]
        al = np.asarray(al, np.float64)[0]
        hh, dd = al.shape
        A = np.zeros((hh * dd, hh), np.float64)
        for h in range(hh):
            A[h * dd:(h + 1) * dd, h] = al[h]
        return A

    W3 = np.asarray(inputs["W3"], np.float64)
    resW3 = np.asarray(inputs["resW3"], np.float64)
    al3 = np.asarray(inputs["al3"], np.float64)[0, :, 0]
    ar3 = np.asarray(inputs["ar3"], np.float64)[0, :, 0]

    shared = {}
    for l in range(3):
        W = np.asarray(inputs[f"W{l}"], np.float64)
        Aal = Amat(inputs[f"al{l}"])
        Aar = Amat(inputs[f"ar{l}"])
        cast = _bf if DENSE_BF else (lambda a: np.asarray(a, np.float32))
        shared[f"W{l}"] = cast(W * (FT_SCALE if PK_FP8 else 1.0))
        shared[f"WA{l}"] = cast(np.concatenate([W @ Aal, W @ Aar], axis=1))
    W3c = np.concatenate(
        [W3, W3 * al3[None, :], W3 * ar3[None, :], resW3], axis=1
    ).astype(np.float32)  # [512, 24]
    b3row = np.zeros((1, 24), np.float32)
    b3row[0, 18:24] = np.asarray(inputs["b3"], np.float32)

    bias_bc = np.tile(
        np.concatenate([np.asarray(inputs[f"b{l}"], np.float32)
                        for l in range(3)])[None, :], (128, 1))  # [128, 3*512]
    lin_bc = np.zeros((128, H3 + 1), np.float32)
    lin_bc[:, 0:H3] = np.asarray(inputs["linW"], np.float32)[:, 0][None, :]
    lin_bc[:, H3] = float(np.asarray(inputs["linb"], np.float32)[0])

    shared.update({
        "W3c": W3c,
        "b3row": b3row,
        "bias_bc": bias_bc,
        "lin_bc": lin_bc,
        "identity": np.eye(128, dtype=np.float32),
        "ones1": np.ones((1, 128), np.float32),
    })

    in_maps = []
    eye64 = np.eye(G, dtype=np.float32)
    for c in range(NCORES):
        blocks = per_core[c]
        idxX = [np.zeros((128, NB * EBS[p] // 16), np.int16) for p in range(P)]
        idxE = np.zeros((128, NB * EBT // 16), np.int16)
        Sarr = np.zeros((NB, 128, EBT), np.float32)
        for b in range(NB):
            epad = np.zeros(EBT, np.int64)
            for p in range(P):
                eb, db = blocks[b][p]
                npd = EBS[p] - len(eb)
                # table-p row id: (owner core)*SHP + (local row within group)
                rows = (eb // SH) * SHP + (eb % SH) - p * SHP
                idxX[p][:, b * EBS[p] // 16:(b + 1) * EBS[p] // 16] = \
                    _wrap_idx(np.concatenate([rows, np.zeros(npd, np.int64)]))
                epad[OFF[p] * 128:OFF[p] * 128 + len(db)] = db
                j = np.arange(len(db))
                S3 = Sarr[b].reshape(128, KBT, 128)
                S3[j % 128, OFF[p] + j // 128, db - b * 128] = 1.0
            idxE[:, b * EBT // 16:(b + 1) * EBT // 16] = _wrap_idx(epad)
        gid = graph_id[c * SH:(c + 1) * SH]
        Gh = eye64[gid].reshape(NB, 128, G)
        im = dict(shared)
        im["feat_sh"] = feat[c * SH:(c + 1) * SH]
        for p in range(P):
            im[f"idxX{p}"] = idxX[p]
        im["idxE"] = idxE
        im["Sarr"] = Sarr.astype(ft_np)
        im["Gh"] = Gh
        in_maps.append(im)
    return in_maps, KBS


@functools.lru_cache(maxsize=4)
def build_program(KBS, ft_dt_name, debug_dump=False, phases=8, nb_lim=NB,
                  l3_lim=3, reps=1):
    FT = {"float32": f32r, "bfloat16": bf16}[ft_dt_name.split("+")[0]]
    ladder = "+lad" in ft_dt_name
    EBS = [k * 128 for k in KBS]
    KBT = sum(KBS)
    EBT = KBT * 128
    OFF = [0]
    for k in KBS:
        OFF.append(OFF[-1] + k)
    nc = bacc.Bacc("TRN2", target_bir_lowering=False, debug=False)

    # ---- I/O ----
    wdt_l = bf16 if DENSE_BF else f32r
    feat_sh = nc.dram_tensor("feat_sh", [SH, F0], f32, kind="ExternalInput")
    Wt = {
        0: nc.dram_tensor("W0", [F0, F], wdt_l, kind="ExternalInput"),
        1: nc.dram_tensor("W1", [F, F], wdt_l, kind="ExternalInput"),
        2: nc.dram_tensor("W2", [F, F], wdt_l, kind="ExternalInput"),
        3: nc.dram_tensor("W3c", [F, 24], f32r, kind="ExternalInput"),
    }
    WAt = {l: nc.dram_tensor(f"WA{l}", [F0 if l == 0 else F, 16], wdt_l,
                             kind="ExternalInput") for l in range(3)}
    b3row = nc.dram_tensor("b3row", [1, 24], f32r, kind="ExternalInput")
    bias_bc = nc.dram_tensor("bias_bc", [128, 3 * F], f32, kind="ExternalInput")
    lin_bc = nc.dram_tensor("lin_bc", [128, H3 + 1], f32, kind="ExternalInput")
    identity = nc.dram_tensor("identity", [128, 128], f32, kind="ExternalInput")
    ones1 = nc.dram_tensor("ones1", [1, 128], f32r, kind="ExternalInput")
    idxXt = {p: nc.dram_tensor(f"idxX{p}", [128, NB * EBS[p] // 16], i16,
                               kind="ExternalInput") for p in range(P)}
    idxE = nc.dram_tensor("idxE", [128, NB * EBT // 16], i16,
                          kind="ExternalInput")
    Sarr = nc.dram_tensor("Sarr", [NB, 128, EBT], FT, kind="ExternalInput")
    Gh = nc.dram_tensor("Gh", [NB, 128, G], f32r, kind="ExternalInput")
    out = nc.dram_tensor("out", [G, 1], f32, kind="ExternalOutput")

    rg = [list(range(NCORES))]

    with tile.TileContext(nc) as tc:
        with (
            tc.tile_pool(name="const", bufs=1) as constp,
            tc.tile_pool(name="wpool", bufs=2) as wpool,
            tc.tile_pool(name="work", bufs=3) as work,
            tc.tile_pool(name="edge", bufs=3) as edge,
            tc.tile_pool(name="erp", bufs=1) as erp,
            tc.tile_pool(name="psA", bufs=2, space="PSUM") as psA,
            tc.tile_pool(name="psB", bufs=2, space="PSUM") as psB,
            tc.tile_pool(name="psC", bufs=2, space="PSUM") as psC,
            tc.tile_pool(name="psP", bufs=1, space="PSUM") as psP,
            tc.tile_pool(name="dram", bufs=1, space="DRAM") as dram,
        ):
            # ---- resident constants ----
            ident_sb = constp.tile([128, 128], f32)
            nc.sync.dma_start(ident_sb[:], identity[:])
            ones_sb = constp.tile([1, 128], f32r)
            nc.sync.dma_start(ones_sb[:], ones1[:])
            b3r_sb = constp.tile([1, 24], f32r)
            nc.sync.dma_start(b3r_sb[:], b3row[:])
            lin_sb = constp.tile([128, H3 + 1], f32)
            nc.sync.dma_start(lin_sb[:], lin_bc[:])
            bias_sb = constp.tile([128, 3 * F], f32)
            nc.sync.dma_start(bias_sb[:], bias_bc[:])
            res3_sb = constp.tile([128, NCH * H3], f32)

            # ---- internal DRAM arrays ----
            # dma_gather needs elem/stride in 256B multiples: the aux block
            # holding el(8) | er(8) | pad is 128 bf16 / 64 f32 elements
            # (256 fp8 elements when the payload is fp8).
            pk8 = "+pk8" in ft_dt_name
            if pk8:
                PKD = fp8
                PAY = 256
            else:
                PKD = FT
                PAY = 128 if FT == bf16 else 64
            FR = F + PAY  # packed row: ft(512) | el(8) er(8) pad
            h_in = {l: dram.tile([SH, F], f32, name=f"h{l}") for l in (1, 2, 3)}
            ftag = {l: dram.tile([SH, FR], PKD, name=f"ftag{l}") for l in range(3)}
            ftg = {(r, l, p): dram.tile([NT, FR], PKD, name=f"ftg{r}_{l}_{p}",
                                        addr_space="Shared")
                   for r in range(reps) for l in range(3) for p in range(P)}
            ft3ag = dram.tile([SH, 64], f32, name="ft3ag")
            ft3g = {(r, p): dram.tile([NT, 64], f32, name=f"ft3g{r}_{p}",
                                      addr_space="Shared")
                    for r in range(reps) for p in range(P)}
            ar_in = dram.tile([G, H3], f32, name="arin")
            ar_out = {r: dram.tile([NCORES, G, H3], f32, name=f"arout{r}",
                                   addr_space="Shared")
                      for r in range(reps)}

            # ================= dense phase =================
            LGROUPS = [[[0, 1], [2, 3], [4, 5], [6, 7]],
                       [[0, 2], [1, 3], [4, 6], [5, 7]],
                       [[0, 4], [1, 5], [2, 6], [3, 7]]]

            def ag8(in_ap, out_t, nm):
                '''8-rank AllGather: single ring or recursive-doubling.'''
                if not ladder:
                    nc.gpsimd.collective_compute(
                        "AllGather", ALU.bypass, replica_groups=rg,
                        ins=[in_ap.opt()], outs=[out_t[:].opt()])
                    return
                rows, cols = in_ap.shape[0], in_ap.shape[1]
                dt_ = in_ap.dtype
                cur = in_ap
                for rnd in range(2):
                    nxt = dram.tile([rows * 2, cols], dt_, name=f"{nm}_r{rnd}")
                    nc.gpsimd.collective_compute(
                        "AllGather", ALU.bypass, replica_groups=LGROUPS[rnd],
                        ins=[cur.opt()], outs=[nxt[:].opt()])
                    cur = nxt[:]
                    rows *= 2
                nc.gpsimd.collective_compute(
                    "AllGather", ALU.bypass, replica_groups=LGROUPS[2],
                    ins=[cur.opt()], outs=[out_t[:].opt()])

            wcache = {}

            def load_w(l):
                K = F0 if l == 0 else F
                KBl = K // 128
                wdt = wdt_l if l < 3 else f32r
                w_sb = wpool.tile([128, KBl, F if l < 3 else 24], wdt, tag="W")
                nc.sync.dma_start(
                    w_sb[:], Wt[l][:].rearrange("(kb p) f -> p kb f", p=128))
                wa_sb = None
                if l < 3:
                    wa_sb = wpool.tile([128, KBl, 16], wdt_l, tag="WA")
                    nc.sync.dma_start(
                        wa_sb[:], WAt[l][:].rearrange("(kb p) f -> p kb f", p=128))
                wcache[l] = (w_sb, wa_sb)

            def dense_chunk(l, j, hl, rep):
                """One 128-node chunk of layer-l dense from an SBUF h tile."""
                K = F0 if l == 0 else F
                KBl = K // 128
                wdt = wdt_l if l < 3 else f32r
                w_sb, wa_sb = wcache[l]
                pT = psB.tile([128, K], f32, tag="T", bufs=1)
                for kb in range(KBl):
                    nc.tensor.transpose(
                        pT[:, kb * 128:(kb + 1) * 128],
                        hl[:, kb * 128:(kb + 1) * 128], ident_sb[:])
                hT = work.tile([128, K], wdt, tag="hT" if l < 3 else "hT3")
                nc.scalar.copy(hT[:], pT[:])
                FW = F if l < 3 else 24
                pft = psB.tile([128, FW], f32, tag="B")
                for kb in range(KBl):
                    nc.tensor.matmul(
                        pft[:], hT[:, kb * 128:(kb + 1) * 128],
                        w_sb[:, kb, :],
                        start=(kb == 0), stop=(kb == KBl - 1 and l < 3))
                if l == 3:
                    nc.tensor.matmul(pft[:], ones_sb[:], b3r_sb[:],
                                     start=False, stop=True)
                if l < 3:
                    pel = psC.tile([128, 16], f32, tag="C")
                    for kb in range(KBl):
                        nc.tensor.matmul(
                            pel[:], hT[:, kb * 128:(kb + 1) * 128],
                            wa_sb[:, kb, :],
                            start=(kb == 0), stop=(kb == KBl - 1))
                    ftt = work.tile([128, FR], PKD, tag="ftsb")
                    nc.scalar.copy(ftt[:, 0:F], pft[:])
                    if pk8:
                        nc.scalar.copy(
                            ftt[:, F:F + 32].bitcast(bf16), pel[:])
                    else:
                        nc.scalar.copy(ftt[:, F:F + 16], pel[:])
                    nc.sync.dma_start(ftag[l][j * 128:(j + 1) * 128, :], ftt[:])
                else:
                    ft3t = work.tile([128, 64], f32, tag="ftsb")
                    nc.any.tensor_copy(ft3t[:, 0:18], pft[:, 0:18])
                    nc.any.tensor_copy(
                        res3_sb[:, j * H3:(j + 1) * H3], pft[:, 18:24])
                    nc.sync.dma_start(ft3ag[j * 128:(j + 1) * 128, :], ft3t[:])
                # group p rows complete -> fire its AllGather half
                if (j + 1) % NCHP == 0:
                    p = (j + 1) // NCHP - 1
                    sl = slice(p * SHP, (p + 1) * SHP)
                    if l < 3:
                        ag8(ftag[l][sl, :], ftg[(rep, l, p)],
                            f"lad{rep}_{l}_{p}")
                    else:
                        ag8(ft3ag[sl, :], ft3g[(rep, p)], f"lad3{rep}_{p}")

            def dense_phase(l, rep=0):
                load_w(l)
                K = F0 if l == 0 else F
                h_src = feat_sh if l == 0 else h_in[l]
                for j in range(NCH):
                    hl = work.tile([128, K], f32, tag="hload")
                    nc.sync.dma_start(hl[:], h_src[j * 128:(j + 1) * 128, :])
                    dense_chunk(l, j, hl, rep)

            # ================= edge phase (layers 0-2) =================
            # Fused: after block b's aggregation produces hn, immediately run
            # the NEXT layer's dense chunk b from SBUF, so dense hides under
            # the edge gathers and the next AllGather fires at block 31.
            def edge_phase(l, rep=0):
                load_w(l + 1)
                # er pre-pass: dst-side er from the LOCAL table; overlaps AG
                er_all = erp.tile([128, NB, KBT, H], FT, tag="erall", bufs=2)
                for b in range(nb_lim):
                    ixeb = edge.tile([128, EBT // 16], i16, tag="ixeb")
                    nc.sync.dma_start(
                        ixeb[:], idxE[:, b * EBT // 16:(b + 1) * EBT // 16])
                    ELt = edge.tile([128, KBT, PAY], PKD, tag="EL")
                    nc.gpsimd.dma_gather(
                        ELt[:], ftag[l][:, F:FR], ixeb[:],
                        num_idxs=EBT, num_idxs_reg=EBT, elem_size=PAY,
                        elem_step=FR, single_packet=False)
                    ers = (ELt[:, 0:KBT, 16:32].bitcast(bf16) if pk8
                           else ELt[:, 0:KBT, H:2 * H])
                    nc.vector.tensor_copy(er_all[:, b], ers)
                for b in range(nb_lim):
                    S_t = edge.tile([128, KBT, 128], FT, tag="S")
                    nc.sync.dma_start(
                        S_t[:], Sarr[b].rearrange("p (c d) -> p c d", d=128))
                    Xs = []
                    for p in range(P):
                        ixb = edge.tile([128, EBS[p] // 16], i16, tag=f"ixb{p}")
                        nc.sync.dma_start(
                            ixb[:],
                            idxXt[p][:, b * EBS[p] // 16:(b + 1) * EBS[p] // 16])
                        X = edge.tile([128, KBS[p], FR], PKD, tag=f"X{p}")
                        nc.gpsimd.dma_gather(
                            X[:], ftg[(rep, l, p)][:], ixb[:],
                            num_idxs=EBS[p], num_idxs_reg=EBS[p], elem_size=FR,
                            single_packet=False)
                        Xs.append(X)
                    et = edge.tile([128, KBT, H], f32, tag="et")
                    for p in range(P):
                        if pk8:
                            elsrc = Xs[p][:, 0:KBS[p], F:F + 16].bitcast(bf16)
                        elsrc = (Xs[p][:, 0:KBS[p], F:F + H] if FT == bf16
                                 else Xs[p][:, 0:KBS[p], F:F + H].bitcast(f32)) \
                            if not pk8 else elsrc
                        nc.vector.tensor_tensor(
                            et[:, OFF[p]:OFF[p + 1]], elsrc,
                            er_all[:, b, OFF[p]:OFF[p + 1]], ALU.add)
                    lt = edge.tile([128, KBT * H], f32, tag="lt")
                    nc.vector.scalar_tensor_tensor(
                        lt[:], et[:].rearrange("p c h -> p (c h)"), NEG_SLOPE,
                        et[:].rearrange("p c h -> p (c h)"), ALU.mult, ALU.max)
                    pt = edge.tile([128, KBT * H], FT, tag="pt")
                    nc.scalar.activation(pt[:], lt[:], AF.Exp)
                    ptb = pt[:] if FT == bf16 else pt[:].bitcast(f32)
                    Ys = []
                    for p in range(P):
                        Xv = Xs[p][:, :, 0:F].rearrange(
                            "p c (h d) -> p c h d", h=H)
                        Xvr = Xv if FT == bf16 or pk8 else Xv.bitcast(f32)
                        pb = ptb.rearrange("p (c h) -> p c h", h=H) \
                            [:, OFF[p]:OFF[p + 1]].unsqueeze(3) \
                            .broadcast_to([128, KBS[p], H, D])
                        if pk8:
                            Y = edge.tile([128, KBS[p], F], FT, tag=f"Y{p}")
                            Yv = Y[:].rearrange("p c (h d) -> p c h d", h=H)
                            nc.vector.tensor_tensor(Yv, Xvr, pb, ALU.mult)
                            Ys.append(Y)
                        else:
                            nc.vector.tensor_tensor(Xv, Xvr, pb, ALU.mult)
                    prst = psA.tile([128, F], f32, tag="A")
                    psw = psC.tile([128, 16], f32, tag="C")
                    ps = psw[:, 0:H]
                    for p in range(P):
                        for c in range(KBS[p]):
                            mov = (Ys[p][:, c, :] if pk8
                                   else Xs[p][:, c, 0:F])
                            nc.tensor.matmul(
                                prst[:], S_t[:, OFF[p] + c, :], mov,
                                start=(OFF[p] + c == 0),
                                stop=(OFF[p] + c == KBT - 1))
                    for c in range(KBT):
                        nc.tensor.matmul(
                            ps, S_t[:, c, :], pt[:, c * H:(c + 1) * H],
                            start=(c == 0), stop=(c == KBT - 1))
                    sse = edge.tile([128, H], f32, tag="sse")
                    nc.vector.tensor_scalar_add(sse[:], ps, EPS)
                    rs = edge.tile([128, H], f32, tag="rs")
                    nc.vector.reciprocal(rs[:], sse[:])
                    t1 = edge.tile([128, H, D], f32, tag="t1")
                    nc.vector.scalar_tensor_tensor(
                        t1[:], prst[:].rearrange("p (h d) -> p h d", h=H),
                        1.0 / FT_SCALE if "+pk8" in ft_dt_name else 1.0,
                        rs[:].unsqueeze(2).broadcast_to([128, H, D]),
                        ALU.mult, ALU.mult)
                    t1f = t1[:].rearrange("p h d -> p (h d)")
                    t2 = edge.tile([128, F], f32, tag="t2")
                    if l == 0:
                        nc.vector.tensor_tensor(
                            t2[:], t1f, bias_sb[:, l * F:(l + 1) * F], ALU.add)
                    else:
                        hres = edge.tile([128, F], f32, tag="hres")
                        nc.sync.dma_start(
                            hres[:], h_in[l][b * 128:(b + 1) * 128, :])
                        t2a = edge.tile([128, F], f32, tag="t2a")
                        nc.gpsimd.tensor_tensor(t2a[:], t1f, hres[:], ALU.add)
                        nc.vector.tensor_tensor(
                            t2[:], t2a[:], bias_sb[:, l * F:(l + 1) * F], ALU.add)
                    # ELU
                    mm = edge.tile([128, F], f32, tag="mm")
                    nc.vector.tensor_scalar_min(mm[:], t2[:], 0.0)
                    ex = edge.tile([128, F], f32, tag="ex")
                    nc.scalar.activation(ex[:], mm[:], AF.Exp)
                    rl = edge.tile([128, F], f32, tag="rl")
                    nc.scalar.activation(rl[:], t2[:], AF.Relu)
                    hn = edge.tile([128, F], f32, tag="hn")
                    nc.vector.scalar_tensor_tensor(
                        hn[:], ex[:], -1.0, rl[:], ALU.add, ALU.add)
                    nc.sync.dma_start(
                        h_in[l + 1][b * 128:(b + 1) * 128, :], hn[:])
                    dense_chunk(l + 1, b, hn, rep)

            # ================= edge phase (layer 3) + pooling ===============
            def edge_phase3(rep=0):
                er3 = erp.tile([128, NB, KBT, H3], f32, tag="er3", bufs=2)
                for b in range(NB):
                    ixeb = edge.tile([128, EBT // 16], i16, tag="ixeb")
                    nc.sync.dma_start(
                        ixeb[:], idxE[:, b * EBT // 16:(b + 1) * EBT // 16])
                    ELt = edge.tile([128, KBT, 64], f32, tag="EL3")
                    nc.gpsimd.dma_gather(
                        ELt[:], ft3ag[:], ixeb[:],
                        num_idxs=EBT, num_idxs_reg=EBT, elem_size=64,
                        single_packet=False)
                    nc.vector.tensor_copy(er3[:, b], ELt[:, 0:KBT, 12:18])
                ppool = psP.tile([G, H3], f32, tag="P")
                for b in range(NB):
                    S_t = edge.tile([128, KBT, 128], FT, tag="S")
                    nc.sync.dma_start(
                        S_t[:], Sarr[b].rearrange("p (c d) -> p c d", d=128))
                    XSs = []
                    for p in range(P):
                        ixb = edge.tile([128, EBS[p] // 16], i16, tag=f"ixb{p}")
                        nc.sync.dma_start(
                            ixb[:],
                            idxXt[p][:, b * EBS[p] // 16:(b + 1) * EBS[p] // 16])
                        XS = edge.tile([128, KBS[p], 64], f32, tag=f"X3{p}")
                        nc.gpsimd.dma_gather(
                            XS[:], ft3g[(rep, p)][:], ixb[:],
                            num_idxs=EBS[p], num_idxs_reg=EBS[p], elem_size=64,
                            single_packet=False)
                        XSs.append(XS)
                    et = edge.tile([128, KBT, H3], f32, tag="et3")
                    for p in range(P):
                        nc.vector.tensor_tensor(
                            et[:, OFF[p]:OFF[p + 1]],
                            XSs[p][:, 0:KBS[p], 6:12],
                            er3[:, b, OFF[p]:OFF[p + 1]], ALU.add)
                    lt = edge.tile([128, KBT * H3], f32, tag="lt3")
                    nc.vector.scalar_tensor_tensor(
                        lt[:], et[:].rearrange("p c h -> p (c h)"), NEG_SLOPE,
                        et[:].rearrange("p c h -> p (c h)"), ALU.mult, ALU.max)
                    XP = edge.tile([128, KBT, 2 * H3], FT, tag="pt3")
                    nc.scalar.activation(
                        XP[:, :, H3:2 * H3],
                        lt[:].rearrange("p (c h) -> p c h", h=H3), AF.Exp)
                    xpr = (XP[:, :, H3:2 * H3] if FT == bf16
                           else XP[:, :, H3:2 * H3].bitcast(f32))
                    for p in range(P):
                        nc.vector.tensor_tensor(
                            XP[:, OFF[p]:OFF[p + 1], 0:H3],
                            XSs[p][:, 0:KBS[p], 0:H3],
                            xpr[:, OFF[p]:OFF[p + 1]], ALU.mult)
                    prstw = psC.tile([128, 16], f32, tag="C")
                    prst = prstw[:, 0:2 * H3]
                    for c in range(KBT):
                        nc.tensor.matmul(
                            prst[:], S_t[:, c, :], XP[:, c, :],
                            start=(c == 0), stop=(c == KBT - 1))
                    sse = edge.tile([128, H3], f32, tag="sse3")
                    nc.vector.tensor_scalar_add(sse[:], prst[:, H3:2 * H3], EPS)
                    rs = edge.tile([128, H3], f32, tag="rs3")
                    nc.vector.reciprocal(rs[:], sse[:])
                    t1 = edge.tile([128, H3], f32, tag="t13")
                    nc.vector.tensor_tensor(t1[:], prst[:, 0:H3], rs[:], ALU.mult)
                    h3 = edge.tile([128, H3], f32r, tag="hn3")
                    nc.vector.tensor_tensor(
                        h3[:], t1[:], res3_sb[:, b * H3:(b + 1) * H3], ALU.add)
                    if l3_lim >= 2:
                        Gt = edge.tile([128, G], f32r, tag="Gt")
                        nc.sync.dma_start(Gt[:], Gh[b])
                        nc.tensor.matmul(
                            ppool[:], Gt[:], h3[:],
                            start=(b == 0), stop=(b == NB - 1))
                if l3_lim < 3:
                    return
                # readout
                pol = work.tile([G, H3], f32, tag="pol")
                nc.any.tensor_copy(pol[:], ppool[:])
                nc.sync.dma_start(ar_in[:], pol[:])
                nc.gpsimd.collective_compute(
                    "AllGather", ALU.bypass, replica_groups=rg,
                    ins=[ar_in[:].opt()], outs=[ar_out[rep][:].opt()])
                pol2 = work.tile([G, NCORES, H3], f32, tag="pol2")
                nc.sync.dma_start(
                    pol2[:], ar_out[rep][:].rearrange("r g h -> g r h"))
                pacc = work.tile([G, H3], f32, tag="pacc")
                nc.vector.tensor_tensor(
                    pacc[:], pol2[:, 0], pol2[:, 1], ALU.add)
                for r2 in range(2, NCORES):
                    nc.vector.tensor_tensor(
                        pacc[:], pacc[:], pol2[:, r2], ALU.add)
                pr = work.tile([G, H3], f32, tag="pr")
                nc.vector.tensor_tensor(pr[:], pacc[:], lin_sb[0:G, 0:H3], ALU.mult)
                ro = work.tile([G, 1], f32, tag="ro")
                nc.vector.tensor_reduce(ro[:], pr[:], axis=AX.X, op=ALU.add)
                ro2 = work.tile([G, 1], f32, tag="ro2")
                nc.vector.tensor_tensor(
                    ro2[:], ro[:], lin_sb[0:G, H3:H3 + 1], ALU.add)
                nc.sync.dma_start(out[:], ro2[:])

            for rep in range(reps):
                steps = [("dense0", lambda r=rep: dense_phase(0, r)),
                         ("edge0d1", lambda r=rep: edge_phase(0, r)),
                         ("edge1d2", lambda r=rep: edge_phase(1, r)),
                         ("edge2d3", lambda r=rep: edge_phase(2, r)),
                         ("edge3", lambda r=rep: edge_phase3(r))]
                for nm, st in steps[:phases]:
                    with nc.named_scope(f"{nm}_{rep}"):
                        st()

    nc.compile()
    return nc


def ft_key():
    return (("float32" if FT_DT == f32 else "bfloat16")
            + ("+dbf" if DENSE_BF else "")
            + ("+pk8" if PK_FP8 else "")
            + ("+lad" if AG_LADDER else "")
            + f"+P{P}")


def kernel(**inputs):
    in_maps, KBS = preprocess(inputs)
    ft_name = ft_key()
    nc = build_program(KBS, ft_name, LAST.get("debug_dump", False),
                       LAST.get("phases", 8), LAST.get("nb_lim", NB),
                       LAST.get("l3_lim", 3))
    br = run_bass_kernel_spmd(
        nc, in_maps, core_ids=list(range(NCORES)), trace=TRACE, **TRACE_KW)
    LAST["br"] = br
    return np.asarray(br.results[0]["out"], dtype=np.float32)
